# revision 2
# baseline (speedup 1.0000x reference)
"""Trainium2 Bass kernel for nn_DiscreteContinuousEncoder (DISCO S2 contraction).

Math (see torch_harmonics _disco_s2_contraction):
    out[b,o,xo,po] = sum_{c,k,n} weight[o,c,k] * psi_vals[k,xo,n]
                     * x[b,c, row(k,xo,n), (col(k,xo,n) + 2*po) mod WIN]

Structure exploited: row(k,xo,n) in [2*xo-2, 2*xo+2] (5-row band, dr in [0,5)),
col(k,xo,n) in [0,16) (cc taps). Host densifies psi+weight into per-latitude
stencils W2[xo, cc, (c,dr), o]; the device computes, per output latitude xo,
16 PSUM-accumulating f32r matmuls with K=(c,xq,dr)=120 (three latitudes packed
block-diagonally along K/M), M=96, N=360x2, where the rhs is a pure
strided view (step 2 = longitude downsampling) into a strip-layout SBUF tile
x_rep[(c,xq,dr), strip] with strip u holding input row 2u+dr.

Sharding: Hout split 8 ways (48 rows/core incl. pad), batch looped on-core.
"""

import os
from contextlib import ExitStack

import numpy as np

import bass_rust

B, CIN, COUT = 4, 8, 32
HIN, WIN = 721, 1440
HOUT, WOUT = 361, 720
KB = 9  # morlet basis
NNZ = 32
PSCALE = WIN // WOUT  # 2

NCORES = 8
XO_PER_CORE = 48          # 8*48 = 384 >= 361 (padded)
NBLK = 2                  # xo blocks per core
XOB = XO_PER_CORE // NBLK  # 24 xo per block
NXQ = 3                   # latitudes packed per matmul (block-diag)
NT = XOB // NXQ           # 8 triples per block
NDR = 5                   # row band
KDIM = CIN * NXQ * NDR    # 120 contraction partitions, p = (c*3+xq)*5+dr
M = NXQ * COUT            # 96
NCC = 16                  # column taps
NH = 2                    # po halves
NPO = WOUT // NH          # 360
WP = WIN + NCC            # 1456 padded columns (wrap)
HR = 2 * XO_PER_CORE + 4  # 100 local rows per core

_CACHE = {}


def _host_prep(x, psi_idx, psi_vals, weight):
    """Slice x per core (+wrap cols, +pad rows) and densify psi into W2."""
    x = np.ascontiguousarray(x, dtype=np.float32)
    psi_idx = np.asarray(psi_idx)
    psi_vals = np.asarray(psi_vals, dtype=np.float32)
    weight = np.asarray(weight, dtype=np.float32)

    rows = psi_idx // WIN          # [KB, HOUT, NNZ]
    cols = psi_idx % WIN
    xo_idx = np.arange(HOUT)
    dr = rows - (2 * xo_idx[None, :, None] - 2)
    assert dr.min() >= 0 and dr.max() < NDR, (dr.min(), dr.max())
    assert cols.max() < NCC, cols.max()

    # S[k, xo, dr, cc] scatter-add of psi_vals
    S = np.zeros((KB, HOUT, NDR, NCC), np.float32)
    k_i = np.repeat(np.arange(KB), HOUT * NNZ)
    xo_i = np.tile(np.repeat(np.arange(HOUT), NNZ), KB)
    np.add.at(
        S, (k_i, xo_i, dr.ravel(), cols.ravel()), psi_vals.ravel()
    )

    # W2d[xo, cc, c, dr, o] = sum_k weight[o,c,k] * S[k,xo,dr,cc]
    W2d = np.einsum("ock,kxdm->xmcdo", weight, S, optimize=True)
    W2d_pad = np.zeros((NCORES * XO_PER_CORE, NCC, CIN, NDR, COUT), np.float32)
    W2d_pad[:HOUT] = W2d

    # per-core weight tensor  [KDIM, NBLK*NT*NCC*M]
    # partition p=(c*3+xq)*5+dr ; free f=((blk*NT+t)*NCC+cc)*M + xq*32 + o
    w2 = np.zeros((NCORES, CIN, NXQ, NDR, NBLK * NT, NCC, NXQ, COUT), np.float32)
    xo_of = (
        np.arange(NCORES)[:, None, None] * XO_PER_CORE
        + (np.arange(NBLK * NT) * NXQ)[None, :, None]
        + np.arange(NXQ)[None, None, :]
    )  # [h, t_global, xq]
    for xq in range(NXQ):
        # w2[h, c, xq, dr, t, cc, xq, o] = W2d_pad[xo, cc, c, dr, o]
        blkv = W2d_pad[xo_of[:, :, xq]]  # [h, tg, cc, c, dr, o]
        w2[:, :, xq, :, :, :, xq, :] = blkv.transpose(0, 3, 4, 1, 2, 5)
    w2 = np.ascontiguousarray(
        w2.reshape(NCORES, KDIM, NBLK * NT * NCC * M)
    )

    # per-core x slices [B, CIN, HR, WP]
    xs = np.zeros((NCORES, B, CIN, HR, WP), np.float32)
    for h in range(NCORES):
        r0 = 2 * (h * XO_PER_CORE) - 2
        lo, hi = max(0, r0), min(HIN, r0 + HR)
        xs[h, :, :, lo - r0 : hi - r0, :WIN] = x[:, :, lo:hi, :]
        xs[h, :, :, lo - r0 : hi - r0, WIN:] = x[:, :, lo:hi, : WP - WIN]
    return xs, w2


def _build(reps=1):
    import concourse.tile as tile
    from concourse import bacc, mybir

    nc = bacc.Bacc("TRN2", target_bir_lowering=False, debug=False,
                   num_devices=NCORES)
    f32r = mybir.dt.float32r
    f32 = mybir.dt.float32

    xs_ap = nc.dram_tensor("xs", [B, CIN, HR, WP], f32r,
                           kind="ExternalInput").ap()
    w2_ap = nc.dram_tensor("w2", [KDIM, NBLK * NT * NCC * M], f32r,
                           kind="ExternalInput").ap()
    out_ap = nc.dram_tensor("out", [B, COUT, XO_PER_CORE, WOUT], f32,
                            kind="ExternalOutput").ap()

    def body(ctx, tc):
        wpool = ctx.enter_context(tc.tile_pool(name="w2p", bufs=1))
        xpool = ctx.enter_context(tc.tile_pool(name="xp", bufs=2))
        spool = ctx.enter_context(tc.tile_pool(name="sp", bufs=3))
        pspool = ctx.enter_context(tc.tile_pool(name="psp", bufs=4, space="PSUM"))

        w2_sb = wpool.tile([KDIM, NBLK * NT * NCC * M], f32r)
        nc.sync.dma_start(w2_sb[:], w2_ap[:])

        def compute(tc):
            for b in range(B):
                for blk in range(NBLK):
                    x_t = xpool.tile([KDIM, NT * WP], f32r, tag="x_t")
                    for c in range(CIN):
                        for xq in range(NXQ):
                            src = xs_ap.copy()
                            src.ap = bass_rust.VecI64Pair(
                                [[WP, NDR], [2 * NXQ * WP, NT], [1, WP]]
                            )
                            src.offset = (
                                (b * CIN + c) * HR + 2 * (XOB * blk) + 2 * xq
                            ) * WP
                            p0 = (c * NXQ + xq) * NDR
                            nc.sync.dma_start(x_t[p0 : p0 + NDR, :], src)
                    for t in range(NT):
                        stage = spool.tile([M, WOUT], f32, tag="stage")
                        for half in range(NH):
                            ps = pspool.tile([M, NPO], f32, tag="ps")
                            for cc in range(NCC):
                                w_of = (((blk * NT + t) * NCC) + cc) * M
                                lhsT = w2_sb[:, w_of : w_of + M]
                                r_of = t * WP + cc + WOUT * half
                                rhs = x_t[:, r_of : r_of + 2 * NPO : 2]
                                nc.tensor.matmul(
                                    ps[:, :], lhsT, rhs,
                                    start=(cc == 0), stop=(cc == NCC - 1),
                                )
                            dst_sl = stage[:, half * NPO : (half + 1) * NPO]
                            if half == 0:
                                nc.vector.tensor_copy(dst_sl, ps[:, :])
                            else:
                                nc.scalar.copy(dst_sl, ps[:, :])
                        dst = out_ap.copy()
                        dst.ap = bass_rust.VecI64Pair(
                            [[WOUT, NXQ], [XO_PER_CORE * WOUT, COUT], [1, WOUT]]
                        )
                        dst.offset = (
                            b * COUT * XO_PER_CORE + XOB * blk + NXQ * t
                        ) * WOUT
                        nc.sync.dma_start(dst, stage[:])

        if reps == 1:
            compute(tc)
        else:
            with tc.For_i(0, reps, 1):
                compute(tc)

    with tile.TileContext(nc) as tc, ExitStack() as ctx:
        body(ctx, tc)
    nc.compile()
    return nc


def _get_nc(reps=1):
    if reps not in _CACHE:
        _CACHE[reps] = _build(reps)
    return _CACHE[reps]


def _run(xs, w2, reps=1):
    from concourse.bass_utils import run_bass_kernel_spmd

    nc = _get_nc(reps)
    in_maps = [{"xs": xs[h], "w2": w2[h]} for h in range(NCORES)]
    res = run_bass_kernel_spmd(
        nc, in_maps, core_ids=list(range(NCORES)), trace=False
    )
    outs = np.stack([res.results[h]["out"] for h in range(NCORES)])
    # [h, B, COUT, XO_PER_CORE, WOUT] -> [B, COUT, HOUT, WOUT]
    full = outs.transpose(1, 2, 0, 3, 4).reshape(
        B, COUT, NCORES * XO_PER_CORE, WOUT
    )
    return np.ascontiguousarray(full[:, :, :HOUT, :], dtype=np.float32)


def kernel(x, psi_idx, psi_vals, weight):
    xs, w2 = _host_prep(x, psi_idx, psi_vals, weight)
    return _run(xs, w2, reps=1)


# revision 5
# speedup vs baseline: 33.1082x; 33.1082x over previous
"""Trainium2 Bass kernel for nn_DiscreteContinuousEncoder (DISCO S2 contraction).

Math (torch_harmonics _disco_s2_contraction, dense form):
    out[b,o,xo,po] = sum_{c,k,n} weight[o,c,k] * psi_vals[k,xo,n]
                     * x[b,c, row(k,xo,n), (col(k,xo,n) + 2*po) mod WIN]

Structure: row(k,xo,n) in [2*xo-2, 2*xo+2] (5-row band, dr in [0,5)),
col(k,xo,n) in [0,16) (cc taps).  Host densifies psi+weight into per-latitude
stencils W2[xo, cc, (c,dr), o] and pre-gathers x into a replicated row-band
layout so the device rhs of every matmul is a pure strided view (step 2 =
longitude downsampling).  Device: per 3-latitude block, 16 PSUM-accumulating
f32r matmuls with K=(c,xq,dr)=120 (3 latitudes packed block-diagonally),
M=96, N=360x2, batch b innermost so 4 consecutive matmuls share the
stationary operand.

Sharding: Hout split 8 ways (48 rows/core incl. pad), batch looped on-core.
"""

from contextlib import ExitStack

import numpy as np

import bass_rust

B, CIN, COUT = 4, 8, 32
HIN, WIN = 721, 1440
HOUT, WOUT = 361, 720
KB = 9
NNZ = 32

NCORES = 8
XO_PER_CORE = 48           # 8*48 = 384 >= 361 (padded)
NXQ = 3                    # latitudes per block (packed along K / M)
NBLK = XO_PER_CORE // NXQ  # 16 blocks of 3 latitudes
NDR = 5                    # row band
KDIM = CIN * NXQ * NDR     # 120, partition p = (c*3+xq)*5+dr
M = NXQ * COUT             # 96, psum partition = xq*32+o
NCC = 16                   # column taps
NH = 2
NPO = WOUT // NH           # 360
WP = WIN + NCC             # 1456 (wrap columns appended)

_CACHE = {}


def _host_prep(x, psi_idx, psi_vals, weight):
    """Densify psi -> W2 stencils; pre-gather x into the strip layout."""
    x = np.ascontiguousarray(x, dtype=np.float32)
    psi_idx = np.asarray(psi_idx)
    psi_vals = np.asarray(psi_vals, dtype=np.float32)
    weight = np.asarray(weight, dtype=np.float32)

    rows = psi_idx // WIN
    cols = psi_idx % WIN
    dr = rows - (2 * np.arange(HOUT)[None, :, None] - 2)
    assert dr.min() >= 0 and dr.max() < NDR, (dr.min(), dr.max())
    assert cols.max() < NCC, cols.max()

    S = np.zeros((KB, HOUT, NDR, NCC), np.float32)
    k_i = np.repeat(np.arange(KB), HOUT * NNZ)
    xo_i = np.tile(np.repeat(np.arange(HOUT), NNZ), KB)
    np.add.at(S, (k_i, xo_i, dr.ravel(), cols.ravel()), psi_vals.ravel())

    # W2d[xo, cc, c, dr, o]
    W2d = np.einsum("ock,kxdm->xmcdo", weight, S, optimize=True)
    W2d_pad = np.zeros((NCORES * XO_PER_CORE, NCC, CIN, NDR, COUT), np.float32)
    W2d_pad[:HOUT] = W2d

    # w2[h]: [KDIM, NBLK*NCC*M]; partition (c,xq,dr); free ((blk*NCC+cc)*M + xq*32 + o)
    w2 = np.zeros((NCORES, CIN, NXQ, NDR, NBLK, NCC, NXQ, COUT), np.float32)
    for xq in range(NXQ):
        xo = (
            np.arange(NCORES)[:, None] * XO_PER_CORE
            + np.arange(NBLK)[None, :] * NXQ
            + xq
        )  # [h, blk]
        # [h, blk, cc, c, dr, o] -> [h, c, dr, blk, cc, o]
        w2[:, :, xq, :, :, :, xq, :] = W2d_pad[xo].transpose(0, 3, 4, 1, 2, 5)
    w2 = np.ascontiguousarray(w2.reshape(NCORES, KDIM, NBLK * NCC * M))

    # x padded: rows -2..768 -> index +2; columns wrapped to WP
    x_pad = np.zeros((B, CIN, 772, WP), np.float32)
    x_pad[:, :, 2 : 2 + HIN, :WIN] = x
    x_pad[:, :, 2 : 2 + HIN, WIN:] = x[:, :, :, : WP - WIN]

    # xs_dev[h]: [NBLK, KDIM, B*WP]; partition p=(c*3+xq)*5+dr holds
    # row 2*(48h+3blk+xq)-2+dr of channel c, for each b.
    c_of = np.repeat(np.arange(CIN), NXQ * NDR)               # [KDIM]
    xq_of = np.tile(np.repeat(np.arange(NXQ), NDR), CIN)
    dr_of = np.tile(np.arange(NDR), CIN * NXQ)
    xs = np.empty((NCORES, NBLK, KDIM, B * WP), np.float32)
    for h in range(NCORES):
        blk_i = np.arange(NBLK)
        # row_idx[blk, p] (into padded rows: +2 offset cancels the -2)
        row_idx = 2 * (48 * h + NXQ * blk_i[:, None] + xq_of[None, :]) + dr_of[None, :]
        gath = x_pad[:, c_of[None, :], row_idx, :]  # [B, NBLK, KDIM, WP]
        # gath: [B, NBLK, KDIM, WP] -> [NBLK, KDIM, B, WP]
        xs[h] = gath.transpose(1, 2, 0, 3).reshape(NBLK, KDIM, B * WP)
    return xs, w2


def _build(reps=1):
    import concourse.tile as tile
    from concourse import bacc, mybir

    nc = bacc.Bacc("TRN2", target_bir_lowering=False, debug=False,
                   num_devices=NCORES)
    f32r = mybir.dt.float32r
    f32 = mybir.dt.float32

    xs_ap = nc.dram_tensor("xs", [NBLK, KDIM, B * WP], f32r,
                           kind="ExternalInput").ap()
    w2_ap = nc.dram_tensor("w2", [KDIM, NBLK * NCC * M], f32r,
                           kind="ExternalInput").ap()
    out_ap = nc.dram_tensor("out", [B, COUT, XO_PER_CORE, WOUT], f32,
                            kind="ExternalOutput").ap()

    def body(ctx, tc):
        wpool = ctx.enter_context(tc.tile_pool(name="w2p", bufs=1))
        xpool = ctx.enter_context(tc.tile_pool(name="xp", bufs=2))
        spool = ctx.enter_context(tc.tile_pool(name="sp", bufs=8))
        pspool = ctx.enter_context(tc.tile_pool(name="psp", bufs=8, space="PSUM"))

        w2_sb = wpool.tile([KDIM, NBLK * NCC * M], f32r)
        nc.sync.dma_start(w2_sb[:], w2_ap[:])

        def compute(tc):
            for blk in range(NBLK):
                x_t = xpool.tile([KDIM, B * WP], f32r, tag="x_t")
                nc.sync.dma_start(x_t[:], xs_ap[blk])
                stages = [
                    spool.tile([M, WOUT], f32, tag="stage", name=f"stage_{blk}_{b}")
                    for b in range(B)
                ]
                for half in range(NH):
                    pss = [
                        pspool.tile([M, NPO], f32, tag="ps",
                                    name=f"ps_{blk}_{half}_{b}")
                        for b in range(B)
                    ]
                    for cc in range(NCC):
                        w_of = (blk * NCC + cc) * M
                        lhsT = w2_sb[:, w_of : w_of + M]
                        for b in range(B):
                            r_of = b * WP + cc + WOUT * half
                            rhs = x_t[:, r_of : r_of + 2 * NPO : 2]
                            nc.tensor.matmul(
                                pss[b][:, :], lhsT, rhs,
                                start=(cc == 0), stop=(cc == NCC - 1),
                            )
                    for b in range(B):
                        dst_sl = stages[b][:, half * NPO : (half + 1) * NPO]
                        if b % 2 == 0:
                            nc.vector.tensor_copy(dst_sl, pss[b][:, :])
                        else:
                            nc.scalar.copy(dst_sl, pss[b][:, :])
                for b in range(B):
                    dst = out_ap.copy()
                    dst.ap = bass_rust.VecI64Pair(
                        [[WOUT, NXQ], [XO_PER_CORE * WOUT, COUT], [1, WOUT]]
                    )
                    dst.offset = (b * COUT * XO_PER_CORE + NXQ * blk) * WOUT
                    nc.sync.dma_start(dst, stages[b][:])

        if reps == 1:
            compute(tc)
        else:
            with tc.For_i(0, reps, 1):
                compute(tc)

    with tile.TileContext(nc) as tc, ExitStack() as ctx:
        body(ctx, tc)
    nc.compile()
    return nc


def _get_nc(reps=1):
    if reps not in _CACHE:
        _CACHE[reps] = _build(reps)
    return _CACHE[reps]


def _run(xs, w2, reps=1):
    from concourse.bass_utils import run_bass_kernel_spmd

    nc = _get_nc(reps)
    in_maps = [{"xs": xs[h], "w2": w2[h]} for h in range(NCORES)]
    res = run_bass_kernel_spmd(
        nc, in_maps, core_ids=list(range(NCORES)), trace=False
    )
    outs = np.stack([res.results[h]["out"] for h in range(NCORES)])
    full = outs.transpose(1, 2, 0, 3, 4).reshape(
        B, COUT, NCORES * XO_PER_CORE, WOUT
    )
    return np.ascontiguousarray(full[:, :, :HOUT, :], dtype=np.float32)


def kernel(x, psi_idx, psi_vals, weight):
    xs, w2 = _host_prep(x, psi_idx, psi_vals, weight)
    return _run(xs, w2, reps=1)
